# revision 19
# baseline (speedup 1.0000x reference)
"""CircleLoss (B=4096, D=128, 512 labels) on 8 Trainium2 NeuronCores.

Strategy (per sharding_hint: shard anchors across cores, replicate embeddings):
  * Host: sort anchors by label. Same-label sets become contiguous column
    ranges [start_a, end_a) of the (sorted) similarity matrix, so all mask
    work on device becomes positional range tests (no label compares, no
    O(B^2) mask tensors anywhere). Host ships O(B) index metadata plus the
    layout-transformed embeddings; all O(B^2) pair work runs on device.
  * Embeddings are pre-scaled by 80^(1/4) so S' = sqrt(80)*S and
      logit_n = max(S', -0.4*sqrt80)^2 - 12.8
      logit_p = min(S' - sqrt80, 0.4*sqrt80)^2 - 12.8
    (the -12.8 folds into host count constants; exact rewrite of the
    reference's gamma/margin algebra).
  * Each core owns 512 anchors (4 row-tiles of 128). Per row-tile:
      - 8 matmul chunks S'[128,512] (f32, PE) straight into a fused custom
        DVE op: range-mask (|col-center|>half keeps complement = negatives),
        clamp+square transform, MaxNeg fill, accum=max -> per-anchor masked
        max. One DVE pass per chunk.
      - positives live in a <=384-wide band (sorted labels): 1 matmul + a
        penalty tile (2 tensor_scalar passes from an iota) + 1 fused pass
        over [start,end) incl. the diagonal; the diagonal's clamped value
        (saturates to 12.8 exactly) is subtracted per-anchor afterwards.
      - exp + row-sum on ACT (bias folds the max shift); masked entries are
        ~-FLT_MAX / -1e30 so exp underflows to exact 0.
      - per-anchor tail: lse_n + lse_p + count consts, stable softplus,
        * valid, reduce -> [1,1] partial sum per core.
  * Host: loss = sum(core partials) / n_valid.
"""

import math

import numpy as np

import concourse.bass as bass
import concourse.bacc as bacc
import concourse.tile as tile
from concourse import mybir
import concourse.dve_ops as dve_ops
from concourse.dve_ops import DveOp
from concourse.dve_spec import (
    C0,
    C1,
    C2,
    AluOp,
    Bin,
    MaxNeg,
    Spec,
    Src0,
    Src1,
    _has_src1 as has_src1,
    lower,
    maxx,
    minn,
    select,
    sq,
)
from concourse.dve_uop import DveOpSpec
from concourse.bass_utils import run_bass_kernel_spmd

F32 = mybir.dt.float32
F16 = mybir.dt.float16
AF = mybir.ActivationFunctionType
ALU = mybir.AluOpType

B = 4096
D = 128
P = 128
CH = 512           # matmul sub-chunk (one PSUM bank of f32)
CHW = 1024         # DVE chunk width (2 PSUM banks)
NCH = B // CHW     # 4 DVE chunks per row
RT = 4             # row tiles per core
NCORES = 8
APC = P * RT       # anchors per core = 512
BW_CHOICES = (256, 384, 512)  # positive band width (adaptive per input)
SQRT80 = float(np.float32(np.sqrt(np.float32(80.0))))
SCALE_E = float(np.float32(80.0) ** 0.25)
CLAMP_P = float(np.float32(0.4) * np.float32(SQRT80))
CLAMP_N = float(np.float32(-0.4) * np.float32(SQRT80))
FMIN = float(np.finfo(np.float32).min)
PEN = -1.0e30

# ---------------------------------------------------------------------------
# Custom DVE ops (fused range-mask + clamp + square + max-reduce)
# ---------------------------------------------------------------------------


def _ref_circle_neg(in0, in1, s0, s1, imm2):
    # in0=[P,N] S' chunk; in1=[P,N] iota (column index); s0=center_rel;
    # s1=half; imm2=-0.4*sqrt80 clamp. Keeps complement of the group range.
    p = in0.shape[0]
    x = in0.astype(np.float32).reshape(p, -1)
    idx = np.asarray(in1, np.float32).reshape(p, -1)
    c0 = np.broadcast_to(np.asarray(s0, np.float32).reshape(-1, 1), (p, 1))
    c1 = np.broadcast_to(np.asarray(s1, np.float32).reshape(-1, 1), (p, 1))
    m = np.abs(idx - c0) > c1
    val = np.maximum(x, np.float32(imm2)) ** 2
    body = np.where(m, val, np.float32(FMIN)).astype(np.float32)
    return body, body.max(axis=-1, keepdims=True)


def _ref_circle_pos(in0, in1, s0, s1, imm2):
    # in0=[P,N] S' band; in1=[P,N] additive penalty (0 in range, -1e30 out);
    # s1=sqrt80 shift; imm2=0.4*sqrt80 clamp.
    p = in0.shape[0]
    x = in0.astype(np.float32).reshape(p, -1)
    pen = np.asarray(in1, np.float32).reshape(p, -1)
    val = np.minimum(x - np.float32(s1), np.float32(imm2)) ** 2
    body = (val + pen).astype(np.float32)
    return body, body.max(axis=-1, keepdims=True)


_body_neg = select(
    Bin(AluOp.ABSOLUTE_DIFF, Src1, C0) > C1, sq(maxx(Src0, C2)), MaxNeg
)
_body_pos = sq(minn(Src0 - C1, C2)) + Src1

CIRCLE_NEG = DveOp(
    "CIRCLE_NEG",
    Spec(body=_body_neg, accum=maxx, reference=_ref_circle_neg),
    subdim=False,
    uops_sha={},
)
CIRCLE_POS = DveOp(
    "CIRCLE_POS",
    Spec(body=_body_pos, accum=maxx, reference=_ref_circle_pos),
    subdim=False,
    uops_sha={},
)


def _register(op: DveOp) -> None:
    if op.name in dve_ops._SUB_OPCODE_FOR_NAME:
        return
    dve_ops.OPS.append(op)
    dve_ops._SUB_OPCODE_FOR_NAME[op.name] = (
        max(dve_ops._SUB_OPCODE_FOR_NAME.values()) + 1
    )
    assert dve_ops._SUB_OPCODE_FOR_NAME[op.name] < 0x20
    dve_ops.CUSTOM_DVE_SPECS[op.name] = op.spec
    for ver in ("v3", "v4"):
        spec_c = DveOpSpec(
            name=op.name,
            opcode=dve_ops._SUB_OPCODE_FOR_NAME[op.name],
            uops=lower(op.spec, ver=ver),
            rd1_en=has_src1(op.spec),
        )
        op.uops_sha[ver] = spec_c.sha(ver)


_register(CIRCLE_NEG)
_register(CIRCLE_POS)


# ---------------------------------------------------------------------------
# Device program (one core's 512 anchors; SPMD — per-core differences are data)
# ---------------------------------------------------------------------------

# meta columns (f32, [APC, 24]):
#   0..3 : neg center_rel(c) = (start+end-1)/2 - 1024*c
#   8    : half = (end-start-1)/2
#   16   : start_band = start - band_start
#   17   : end_band = end - band_start
#   18   : diag_sq = min(S'_aa - sqrt80, 0.4*sqrt80)^2  (host, f32)
#   20   : log(max(p_cnt,1)) + log(max(n_cnt,1)) - 25.6
#   21   : valid (0/1)
MCOLS = 24


def build_program(BW=384, bench_iters=1):
    nc = bacc.Bacc("TRN2", target_bir_lowering=False, debug=False)
    # split-fp16 embeddings: e = h + l exactly captures f32 to ~2^-22;
    # S = h.h + h.l + l.h (the dropped l.l term is ~2^-22 relative).
    eth = nc.dram_tensor("eth", [P, B], F16, kind="ExternalInput")
    etl = nc.dram_tensor("etl", [P, B], F16, kind="ExternalInput")
    eah = nc.dram_tensor("eah", [P, APC], F16, kind="ExternalInput")
    eal = nc.dram_tensor("eal", [P, APC], F16, kind="ExternalInput")
    ebh = nc.dram_tensor("ebh", [P, RT * BW], F16, kind="ExternalInput")
    ebl = nc.dram_tensor("ebl", [P, RT * BW], F16, kind="ExternalInput")
    meta = nc.dram_tensor("meta", [APC, MCOLS], F32, kind="ExternalInput")
    out = nc.dram_tensor("out", [1, 1], F32, kind="ExternalOutput")

    with tile.TileContext(nc) as tc:
        with (
            tc.tile_pool(name="singles", bufs=1) as singles,
            tc.tile_pool(name="small", bufs=1) as small,
            tc.tile_pool(name="bandp", bufs=2) as bandp,
            tc.tile_pool(name="esc", bufs=2) as escp,
            tc.tile_pool(name="psum_s", bufs=3, space="PSUM") as psum_s,
            tc.tile_pool(name="psum_f", bufs=1, space="PSUM") as psum_f,
        ):
            eth_sb = singles.tile([P, B], F16)
            etl_sb = singles.tile([P, B], F16)
            eah_sb = singles.tile([P, APC], F16)
            eal_sb = singles.tile([P, APC], F16)
            ebh_sb = singles.tile([P, RT * BW], F16)
            ebl_sb = singles.tile([P, RT * BW], F16)
            meta_sb = singles.tile([P, RT, MCOLS], F32)
            sqm_all = singles.tile([P, RT, B], F32)
            iota_sb = singles.tile([P, CHW], F32)
            ones = singles.tile([P, 1], F32)

            mxn4 = small.tile([P, RT, NCH], F32)
            mxp1 = small.tile([P, RT], F32)
            sn4 = small.tile([P, RT], F32)
            sp4 = small.tile([P, RT], F32)
            biasn = small.tile([P, RT], F32)
            biasp = small.tile([P, RT], F32)
            ed4 = small.tile([P, RT], F32)

            nc.sync.dma_start(out=eth_sb[:], in_=eth[:])
            nc.sync.dma_start(out=etl_sb[:], in_=etl[:])
            nc.sync.dma_start(out=eah_sb[:], in_=eah[:])
            nc.sync.dma_start(out=eal_sb[:], in_=eal[:])
            nc.sync.dma_start(out=ebh_sb[:], in_=ebh[:])
            nc.sync.dma_start(out=ebl_sb[:], in_=ebl[:])
            nc.sync.dma_start(
                out=meta_sb[:], in_=meta.rearrange("(r p) k -> p r k", p=P)
            )
            nc.vector.memset(ones, 1.0)
            nc.gpsimd.iota(
                iota_sb[:], [[1, CHW]], base=0, channel_multiplier=0,
                allow_small_or_imprecise_dtypes=True,
            )

            import contextlib
            loop_cm = (
                tc.For_i(0, bench_iters, 1) if bench_iters > 1
                else contextlib.nullcontext()
            )
            with loop_cm:
              for rt in range(RT):
                mrt = meta_sb[:, rt]
                lh = eah_sb[:, rt * P:(rt + 1) * P]
                ll = eal_sb[:, rt * P:(rt + 1) * P]
                # --- positives first: penalty tile on (otherwise idle) GPSIMD
                pen1 = bandp.tile([P, BW], F32, tag="pen1")
                pen2 = bandp.tile([P, BW], F32, tag="pen2")
                pen = bandp.tile([P, BW], F32, tag="pen")
                nc.gpsimd.tensor_scalar(
                    out=pen1[:], in0=iota_sb[:, :BW],
                    scalar1=mrt[:, 16:17], scalar2=PEN,
                    op0=ALU.is_lt, op1=ALU.mult,
                )
                nc.gpsimd.tensor_scalar(
                    out=pen2[:], in0=iota_sb[:, :BW],
                    scalar1=mrt[:, 17:18], scalar2=PEN,
                    op0=ALU.is_ge, op1=ALU.mult,
                )
                nc.gpsimd.tensor_add(pen[:], pen1[:], pen2[:])
                pbt = psum_s.tile([P, CHW], F32, tag="ps")
                pb = pbt[:, :BW]
                nc.tensor.matmul(
                    pb, lh, ebh_sb[:, rt * BW:(rt + 1) * BW],
                    start=True, stop=False,
                )
                nc.tensor.matmul(
                    pb, lh, ebl_sb[:, rt * BW:(rt + 1) * BW],
                    start=False, stop=False,
                )
                nc.tensor.matmul(
                    pb, ll, ebh_sb[:, rt * BW:(rt + 1) * BW],
                    start=False, stop=True,
                )
                bp = bandp.tile([P, BW], F32, tag="bp")
                nc.vector._custom_dve(
                    CIRCLE_POS,
                    out=bp[:], in0=pb, in1=pen[:],
                    s1=SQRT80, imm2=CLAMP_P,
                    accum_out=mxp1[:, rt:rt + 1],
                )
                # --- negatives: full row in 4-bank superchunks, term-major
                # matmul order so LDWEIGHTS switches only twice per chunk
                nhalves = CHW // CH
                for c in range(NCH):
                    ps = psum_s.tile([P, CHW], F32, tag="ps")
                    for ti, (w, rhs_sb) in enumerate(
                        [(lh, eth_sb), (lh, etl_sb), (ll, eth_sb)]
                    ):
                        for half in range(nhalves):
                            cs = c * CHW + half * CH
                            dst = ps[:, half * CH:(half + 1) * CH]
                            nc.tensor.matmul(
                                dst, w, rhs_sb[:, cs:cs + CH],
                                start=(ti == 0), stop=(ti == 2),
                            )
                    nc.vector._custom_dve(
                        CIRCLE_NEG,
                        out=sqm_all[:, rt, c * CHW:(c + 1) * CHW],
                        in0=ps[:],
                        in1=iota_sb[:],
                        s0=mrt[:, c:c + 1],
                        s1=mrt[:, 8:9],
                        imm2=CLAMP_N,
                        accum_out=mxn4[:, rt, c:c + 1],
                    )

                # --- per-row-tile shifts (tiny) so ACT can start
                nc.vector.tensor_reduce(
                    biasn[:, rt:rt + 1], mxn4[:, rt], axis=mybir.AxisListType.X,
                    op=ALU.max, negate=True,
                )
                nc.vector.tensor_scalar_mul(
                    biasp[:, rt:rt + 1], mxp1[:, rt:rt + 1], -1.0
                )
                # --- exp + row sums (ACT, one full-row pass); masked -> 0
                esc = escp.tile([P, B], F32, tag="esc")
                nc.scalar.activation(
                    out=esc[:], in_=sqm_all[:, rt],
                    func=AF.Exp, bias=biasn[:, rt:rt + 1], scale=1.0,
                    accum_out=sn4[:, rt:rt + 1],
                )
                escb = escp.tile([P, BW], F32, tag="escb")
                nc.scalar.activation(
                    out=escb[:], in_=bp[:],
                    func=AF.Exp, bias=biasp[:, rt:rt + 1], scale=1.0,
                    accum_out=sp4[:, rt:rt + 1],
                )
                # diagonal's exp contribution (to subtract later)
                nc.scalar.activation(
                    out=ed4[:, rt:rt + 1], in_=mrt[:, 18:19],
                    func=AF.Exp, bias=biasp[:, rt:rt + 1], scale=1.0,
                )

            # ---- batched per-anchor tail on [P, RT] tiles (all tiny)
            lnn = small.tile([P, RT], F32)
            nc.vector.tensor_scalar_add(sn4[:], sn4[:], 1e-30)
            nc.scalar.activation(out=lnn[:], in_=sn4[:], func=AF.Ln)

            # positives: subtract diagonal, guard at 0, log
            nc.vector.tensor_sub(sp4[:], sp4[:], ed4[:])
            nc.vector.tensor_scalar(
                out=sp4[:], in0=sp4[:], scalar1=0.0, scalar2=1e-30,
                op0=ALU.max, op1=ALU.add,
            )
            lnp = small.tile([P, RT], F32)
            nc.scalar.activation(out=lnp[:], in_=sp4[:], func=AF.Ln)

            z = small.tile([P, RT], F32)
            # z = (mxn + mxp) + (lnn + lnp) + cnt ; mx* = -bias*
            nc.vector.tensor_add(z[:], biasn[:], biasp[:])
            nc.vector.tensor_scalar_mul(z[:], z[:], -1.0)
            nc.vector.tensor_add(z[:], z[:], lnn[:])
            nc.vector.tensor_add(z[:], z[:], lnp[:])
            nc.vector.tensor_add(z[:], z[:], meta_sb[:, :, 20])

            # stable softplus: relu(z) + softplus(-|z|) (one ACT table func)
            rl = small.tile([P, RT], F32)
            nc.gpsimd.tensor_scalar_max(rl[:], z[:], 0.0)
            negz = small.tile([P, RT], F32)
            nc.gpsimd.tensor_scalar_mul(negz[:], z[:], -1.0)
            ab = small.tile([P, RT], F32)
            nc.vector.tensor_max(ab[:], z[:], negz[:])
            # exp(-|z|) via Schraudolph bit trick on DVE (avoids an ACT
            # Exp<->Ln table reload; |error| ~3% of a term bounded by ln 2,
            # and +inf/huge |z| map to exactly 0)
            ey = small.tile([P, RT], F32)
            nc.vector.tensor_scalar(
                out=ey[:], in0=ab[:], scalar1=-12102203.0, scalar2=1064866805.0,
                op0=ALU.mult, op1=ALU.add,
            )
            nc.vector.tensor_scalar_max(ey[:], ey[:], 0.0)
            eyi = small.tile([P, RT], mybir.dt.int32)
            nc.vector.tensor_copy(eyi[:], ey[:])
            enx = small.tile([P, RT], F32)
            nc.vector.tensor_scalar_add(enx[:], eyi[:].bitcast(F32), 1.0)
            l1p = small.tile([P, RT], F32)
            nc.scalar.activation(out=l1p[:], in_=enx[:], func=AF.Ln)
            sp = small.tile([P, RT], F32)
            nc.vector.tensor_add(sp[:], rl[:], l1p[:])
            nc.vector.tensor_mul(sp[:], sp[:], meta_sb[:, :, 21])

            tot = small.tile([P, 1], F32)
            nc.vector.tensor_reduce(
                tot[:], sp[:], axis=mybir.AxisListType.X, op=ALU.add
            )
            pf = psum_f.tile([1, 1], F32, tag="pf")
            nc.tensor.matmul(pf[:], tot[:], ones[:], start=True, stop=True)
            osb = small.tile([1, 1], F32)
            nc.vector.tensor_copy(osb[:], pf[:])
            nc.sync.dma_start(out=out[:], in_=osb[:])

    nc.compile()
    return nc


# ---------------------------------------------------------------------------
# Host side
# ---------------------------------------------------------------------------


def host_prep(E, labels, batch_size):
    order = np.argsort(labels, kind="stable")
    labels_s = labels[order]
    idx = np.arange(B)
    keep = ((idx % 4 == 0) & (idx < batch_size)) | (idx > batch_size)
    keep_s = keep[order]

    change = np.empty(B, bool)
    change[0] = True
    change[1:] = labels_s[1:] != labels_s[:-1]
    firsts = np.flatnonzero(change)
    bounds = np.concatenate([firsts, [B]])
    start = np.repeat(bounds[:-1], np.diff(bounds))
    end = np.repeat(bounds[1:], np.diff(bounds))

    gsize = end - start
    p_cnt = gsize - 1
    n_cnt = B - gsize
    valid = keep_s & (p_cnt > 0) & (n_cnt > 0)
    cnt = (
        np.log(np.maximum(p_cnt, 1)) + np.log(np.maximum(n_cnt, 1)) - 25.6
    ).astype(np.float32)
    n_valid = int(valid.sum())

    # pre-scaled, transposed embeddings [D, B]
    E_T = np.ascontiguousarray(
        E[order].T * np.float32(SCALE_E), dtype=np.float32
    )
    return E_T, start, end, valid, cnt, n_valid


def band_width_needed(start, end):
    w = 0
    for core in range(NCORES):
        a0 = core * APC
        for rt in range(RT):
            lo = int(start[a0 + rt * P])
            hi = int(end[a0 + rt * P + P - 1])
            w = max(w, hi - lo)
    for bw in BW_CHOICES:
        if w <= bw:
            return bw
    raise ValueError(f"band width {w} exceeds {BW_CHOICES[-1]}")


def make_core_inputs(E_T, start, end, valid, cnt, core, BW):
    a0 = core * APC
    st = start[a0:a0 + APC]
    en = end[a0:a0 + APC]

    center = (st + en - 1).astype(np.float64) / 2.0
    half = (en - st - 1).astype(np.float64) / 2.0

    band_start = np.zeros(APC, np.int64)
    for rt in range(RT):
        lo = int(st[rt * P])
        hi = int(en[rt * P + P - 1])
        if hi - lo > BW:
            raise ValueError(f"band width {hi - lo} exceeds {BW}")
        bs = min(max(lo, 0), B - BW)
        band_start[rt * P:(rt + 1) * P] = bs

    # diagonal value: S'_aa = sum of scaled e^2 (f32), clamped transform
    ecols = E_T[:, a0:a0 + APC].astype(np.float32)
    s_aa = np.sum(ecols * ecols, axis=0, dtype=np.float32)
    dsq = np.minimum(
        s_aa - np.float32(SQRT80), np.float32(CLAMP_P)
    ).astype(np.float32) ** 2

    meta = np.zeros((APC, MCOLS), np.float32)
    for c in range(NCH):
        meta[:, c] = (center - CHW * c).astype(np.float32)
    meta[:, 8] = half.astype(np.float32)
    meta[:, 16] = (st - band_start).astype(np.float32)
    meta[:, 17] = (en - band_start).astype(np.float32)
    meta[:, 18] = dsq
    meta[:, 20] = cnt[a0:a0 + APC]
    meta[:, 21] = valid[a0:a0 + APC].astype(np.float32)

    eb = np.empty((P, RT * BW), np.float32)
    for rt in range(RT):
        bs = int(band_start[rt * P])
        eb[:, rt * BW:(rt + 1) * BW] = E_T[:, bs:bs + BW]

    def split16(a):
        h = a.astype(np.float16)
        l = (a - h.astype(np.float32)).astype(np.float16)
        return h, l

    eth, etl = split16(E_T)
    ea = np.ascontiguousarray(E_T[:, a0:a0 + APC])
    eah, eal = split16(ea)
    ebh, ebl = split16(eb)

    return {
        "eth": eth,
        "etl": etl,
        "eah": eah,
        "eal": eal,
        "ebh": ebh,
        "ebl": ebl,
        "meta": meta,
    }


_PROGRAM_CACHE = {}


def _get_program(BW=384):
    key = ("nc", BW)
    if key not in _PROGRAM_CACHE:
        _PROGRAM_CACHE[key] = build_program(BW)
    return _PROGRAM_CACHE[key]


def _build_executor(nc, n_cores=NCORES):
    """Persistent jitted runner (mirrors bass2jax.run_bass_via_pjrt's
    multi-core branch) so repeated kernel() calls skip jax re-tracing."""
    import jax
    from jax.experimental.shard_map import shard_map
    from jax.sharding import Mesh, PartitionSpec
    from concourse import bass2jax
    from concourse import mybir as _mb

    bass2jax.install_neuronx_cc_hook()
    partition_name = (
        nc.partition_id_tensor.name if nc.partition_id_tensor else None
    )
    in_names, out_names, out_avals, zero_templates = [], [], [], []
    for alloc in nc.m.functions[0].allocations:
        if not isinstance(alloc, _mb.MemoryLocationSet):
            continue
        name = alloc.memorylocations[0].name
        if alloc.kind == "ExternalInput":
            if name != partition_name:
                in_names.append(name)
        elif alloc.kind == "ExternalOutput":
            shape = tuple(alloc.tensor_shape)
            dtype = _mb.dt.np(alloc.dtype)
            out_names.append(name)
            out_avals.append(jax.core.ShapedArray(shape, dtype))
            zero_templates.append((shape, dtype))
    n_params = len(in_names)
    n_outs = len(out_avals)
    all_names = list(in_names) + list(out_names)
    if partition_name is not None:
        all_names.append(partition_name)
    donate = tuple(range(n_params, n_params + n_outs))

    def _body(*args):
        operands = list(args)
        if partition_name is not None:
            operands.append(bass2jax.partition_id_tensor())
        outs = bass2jax._bass_exec_p.bind(
            *operands,
            out_avals=tuple(out_avals),
            in_names=tuple(all_names),
            out_names=tuple(out_names),
            lowering_input_output_aliases=(),
            sim_require_finite=True,
            sim_require_nnan=True,
            nc=nc,
        )
        return tuple(outs)

    devices = jax.devices()[:n_cores]
    mesh = Mesh(np.asarray(devices), ("core",))
    # "et" is identical on every core -> replicate instead of concatenating
    # 8 copies (saves ~14MB of host->device traffic per call).
    replicated = {"eth", "etl"}
    in_specs = tuple(
        PartitionSpec() if name in replicated else PartitionSpec("core")
        for name in in_names
    ) + (PartitionSpec("core"),) * n_outs
    out_specs = (PartitionSpec("core"),) * n_outs
    sharded = jax.jit(
        shard_map(_body, mesh=mesh, in_specs=in_specs, out_specs=out_specs,
                  check_rep=False),
        donate_argnums=donate, keep_unused=True,
    )

    from jax.sharding import NamedSharding

    def place(in_maps):
        """Device-put the call inputs once; returns the positional arg list
        (without the donated zero buffers)."""
        arrs = []
        for name in in_names:
            if name in replicated:
                a = np.asarray(in_maps[0][name])
                sh = NamedSharding(mesh, PartitionSpec())
            else:
                a = np.concatenate(
                    [np.asarray(m[name]) for m in in_maps], axis=0
                )
                sh = NamedSharding(mesh, PartitionSpec("core"))
            arrs.append(jax.device_put(a, sh))
        return arrs

    zero_sharding = NamedSharding(mesh, PartitionSpec("core"))

    def exec_async(dev_in):
        concat_zeros = [
            jax.device_put(np.zeros((n_cores * s[0], *s[1:]), dt), zero_sharding)
            for s, dt in zero_templates
        ]
        return sharded(*dev_in, *concat_zeros)

    def run(in_maps):
        out_arrs = exec_async(place(in_maps))
        return [
            {
                name: np.asarray(out_arrs[i]).reshape(n_cores, *out_avals[i].shape)[c]
                for i, name in enumerate(out_names)
            }
            for c in range(n_cores)
        ]

    run.place = place
    run.exec_async = exec_async
    return run


def _get_executor(BW=384):
    key = ("exec", BW)
    if key not in _PROGRAM_CACHE:
        nc = _get_program(BW)
        try:
            _PROGRAM_CACHE[key] = _build_executor(nc)
        except Exception:
            _PROGRAM_CACHE[key] = None
    return _PROGRAM_CACHE[key]


def _run_device(in_maps, BW=384):
    ex = _get_executor(BW)
    if ex is not None:
        try:
            return ex(in_maps)
        except Exception:
            _PROGRAM_CACHE[("exec", BW)] = None
    res = run_bass_kernel_spmd(
        _get_program(BW), in_maps, core_ids=list(range(NCORES))
    )
    return res.results


def make_all_inputs(embeddings, labels, batch_size):
    E = np.asarray(embeddings, np.float32)
    labels_np = np.asarray(labels).astype(np.int64).reshape(-1)
    bs = int(np.asarray(batch_size).reshape(()))
    assert E.shape == (B, D)
    E_T, start, end, valid, cnt, n_valid = host_prep(E, labels_np, bs)
    BW = band_width_needed(start, end)
    in_maps = [
        make_core_inputs(E_T, start, end, valid, cnt, c, BW)
        for c in range(NCORES)
    ]
    return in_maps, n_valid, BW


def kernel(embeddings, labels, batch_size):
    in_maps, n_valid, BW = make_all_inputs(embeddings, labels, batch_size)
    results = _run_device(in_maps, BW)
    partials = [float(r["out"][0, 0]) for r in results]
    loss = np.float32(math.fsum(partials) / max(n_valid, 1))
    return np.asarray(loss, dtype=np.float32)


# revision 20
# speedup vs baseline: 1.0151x; 1.0151x over previous
"""CircleLoss (B=4096, D=128, 512 labels) on 8 Trainium2 NeuronCores.

Strategy (per sharding_hint: shard anchors across cores, replicate embeddings):
  * Host: sort anchors by label. Same-label sets become contiguous column
    ranges [start_a, end_a) of the (sorted) similarity matrix, so all mask
    work on device becomes positional range tests (no label compares, no
    O(B^2) mask tensors anywhere). Host ships O(B) index metadata plus the
    layout-transformed embeddings; all O(B^2) pair work runs on device.
  * Embeddings are pre-scaled by 80^(1/4) so S' = sqrt(80)*S and
      logit_n = max(S', -0.4*sqrt80)^2 - 12.8
      logit_p = min(S' - sqrt80, 0.4*sqrt80)^2 - 12.8
    (the -12.8 folds into host count constants; exact rewrite of the
    reference's gamma/margin algebra).
  * Each core owns 512 anchors (4 row-tiles of 128). Per row-tile:
      - 8 matmul chunks S'[128,512] (f32, PE) straight into a fused custom
        DVE op: range-mask (|col-center|>half keeps complement = negatives),
        clamp+square transform, MaxNeg fill, accum=max -> per-anchor masked
        max. One DVE pass per chunk.
      - positives live in a <=384-wide band (sorted labels): 1 matmul + a
        penalty tile (2 tensor_scalar passes from an iota) + 1 fused pass
        over [start,end) incl. the diagonal; the diagonal's clamped value
        (saturates to 12.8 exactly) is subtracted per-anchor afterwards.
      - exp + row-sum on ACT (bias folds the max shift); masked entries are
        ~-FLT_MAX / -1e30 so exp underflows to exact 0.
      - per-anchor tail: lse_n + lse_p + count consts, stable softplus,
        * valid, reduce -> [1,1] partial sum per core.
  * Host: loss = sum(core partials) / n_valid.
"""

import math

import numpy as np

import concourse.bass as bass
import concourse.bacc as bacc
import concourse.tile as tile
from concourse import mybir
import concourse.dve_ops as dve_ops
from concourse.dve_ops import DveOp
from concourse.dve_spec import (
    C0,
    C1,
    C2,
    AluOp,
    Bin,
    MaxNeg,
    Spec,
    Src0,
    Src1,
    _has_src1 as has_src1,
    lower,
    maxx,
    minn,
    select,
    sq,
)
from concourse.dve_uop import DveOpSpec
from concourse.bass_utils import run_bass_kernel_spmd

F32 = mybir.dt.float32
F16 = mybir.dt.float16
AF = mybir.ActivationFunctionType
ALU = mybir.AluOpType

B = 4096
D = 128
P = 128
CH = 512           # matmul sub-chunk (one PSUM bank of f32)
CHW = 1024         # DVE chunk width (2 PSUM banks)
NCH = B // CHW     # 4 DVE chunks per row
RT = 4             # row tiles per core
NCORES = 8
APC = P * RT       # anchors per core = 512
BW_CHOICES = (256, 384, 512)  # positive band width (adaptive per input)
SQRT80 = float(np.float32(np.sqrt(np.float32(80.0))))
SCALE_E = float(np.float32(80.0) ** 0.25)
CLAMP_P = float(np.float32(0.4) * np.float32(SQRT80))
CLAMP_N = float(np.float32(-0.4) * np.float32(SQRT80))
FMIN = float(np.finfo(np.float32).min)
PEN = -1.0e30

# ---------------------------------------------------------------------------
# Custom DVE ops (fused range-mask + clamp + square + max-reduce)
# ---------------------------------------------------------------------------


def _ref_circle_neg(in0, in1, s0, s1, imm2):
    # in0=[P,N] S' chunk; in1=[P,N] iota (column index); s0=center_rel;
    # s1=half; imm2=-0.4*sqrt80 clamp. Keeps complement of the group range.
    p = in0.shape[0]
    x = in0.astype(np.float32).reshape(p, -1)
    idx = np.asarray(in1, np.float32).reshape(p, -1)
    c0 = np.broadcast_to(np.asarray(s0, np.float32).reshape(-1, 1), (p, 1))
    c1 = np.broadcast_to(np.asarray(s1, np.float32).reshape(-1, 1), (p, 1))
    m = np.abs(idx - c0) > c1
    val = np.maximum(x, np.float32(imm2)) ** 2
    body = np.where(m, val, np.float32(FMIN)).astype(np.float32)
    return body, body.max(axis=-1, keepdims=True)


def _ref_circle_pos(in0, in1, s0, s1, imm2):
    # in0=[P,N] S' band; in1=[P,N] additive penalty (0 in range, -1e30 out);
    # s1=sqrt80 shift; imm2=0.4*sqrt80 clamp.
    p = in0.shape[0]
    x = in0.astype(np.float32).reshape(p, -1)
    pen = np.asarray(in1, np.float32).reshape(p, -1)
    val = np.minimum(x - np.float32(s1), np.float32(imm2)) ** 2
    body = (val + pen).astype(np.float32)
    return body, body.max(axis=-1, keepdims=True)


_body_neg = select(
    Bin(AluOp.ABSOLUTE_DIFF, Src1, C0) > C1, sq(maxx(Src0, C2)), MaxNeg
)
_body_pos = sq(minn(Src0 - C1, C2)) + Src1

CIRCLE_NEG = DveOp(
    "CIRCLE_NEG",
    Spec(body=_body_neg, accum=maxx, reference=_ref_circle_neg),
    subdim=False,
    uops_sha={},
)
CIRCLE_POS = DveOp(
    "CIRCLE_POS",
    Spec(body=_body_pos, accum=maxx, reference=_ref_circle_pos),
    subdim=False,
    uops_sha={},
)


def _register(op: DveOp) -> None:
    if op.name in dve_ops._SUB_OPCODE_FOR_NAME:
        return
    dve_ops.OPS.append(op)
    dve_ops._SUB_OPCODE_FOR_NAME[op.name] = (
        max(dve_ops._SUB_OPCODE_FOR_NAME.values()) + 1
    )
    assert dve_ops._SUB_OPCODE_FOR_NAME[op.name] < 0x20
    dve_ops.CUSTOM_DVE_SPECS[op.name] = op.spec
    for ver in ("v3", "v4"):
        spec_c = DveOpSpec(
            name=op.name,
            opcode=dve_ops._SUB_OPCODE_FOR_NAME[op.name],
            uops=lower(op.spec, ver=ver),
            rd1_en=has_src1(op.spec),
        )
        op.uops_sha[ver] = spec_c.sha(ver)


_register(CIRCLE_NEG)
_register(CIRCLE_POS)


# ---------------------------------------------------------------------------
# Device program (one core's 512 anchors; SPMD — per-core differences are data)
# ---------------------------------------------------------------------------

# meta columns (f32, [APC, 24]):
#   0..3 : neg center_rel(c) = (start+end-1)/2 - 1024*c
#   8    : half = (end-start-1)/2
#   16   : start_band = start - band_start
#   17   : end_band = end - band_start
#   18   : diag_sq = min(S'_aa - sqrt80, 0.4*sqrt80)^2  (host, f32)
#   20   : log(max(p_cnt,1)) + log(max(n_cnt,1)) - 25.6
#   21   : valid (0/1)
MCOLS = 24


def build_program(BW=384, bench_iters=1):
    nc = bacc.Bacc("TRN2", target_bir_lowering=False, debug=False)
    # split-fp16 embeddings: e = h + l exactly captures f32 to ~2^-22;
    # S = h.h + h.l + l.h (the dropped l.l term is ~2^-22 relative).
    eth = nc.dram_tensor("eth", [P, B], F16, kind="ExternalInput")
    etl = nc.dram_tensor("etl", [P, B], F16, kind="ExternalInput")
    eah = nc.dram_tensor("eah", [P, APC], F16, kind="ExternalInput")
    eal = nc.dram_tensor("eal", [P, APC], F16, kind="ExternalInput")
    ebh = nc.dram_tensor("ebh", [P, RT * BW], F16, kind="ExternalInput")
    ebl = nc.dram_tensor("ebl", [P, RT * BW], F16, kind="ExternalInput")
    meta = nc.dram_tensor("meta", [APC, MCOLS], F32, kind="ExternalInput")
    out = nc.dram_tensor("out", [1, 1], F32, kind="ExternalOutput")

    with tile.TileContext(nc) as tc:
        with (
            tc.tile_pool(name="singles", bufs=1) as singles,
            tc.tile_pool(name="small", bufs=1) as small,
            tc.tile_pool(name="bandp", bufs=2) as bandp,
            tc.tile_pool(name="esc", bufs=2) as escp,
            tc.tile_pool(name="psum_s", bufs=3, space="PSUM") as psum_s,
            tc.tile_pool(name="psum_f", bufs=1, space="PSUM") as psum_f,
        ):
            eth_sb = singles.tile([P, B], F16)
            etl_sb = singles.tile([P, B], F16)
            eah_sb = singles.tile([P, APC], F16)
            eal_sb = singles.tile([P, APC], F16)
            ebh_sb = singles.tile([P, RT * BW], F16)
            ebl_sb = singles.tile([P, RT * BW], F16)
            meta_sb = singles.tile([P, RT, MCOLS], F32)
            sqm_all = singles.tile([P, RT, B], F32)
            iota_sb = singles.tile([P, CHW], F32)
            ones = singles.tile([P, 1], F32)

            mxn4 = small.tile([P, RT, NCH], F32)
            sn2 = small.tile([P, RT, 2], F32)
            mxp1 = small.tile([P, RT], F32)
            sn4 = small.tile([P, RT], F32)
            sp4 = small.tile([P, RT], F32)
            biasn = small.tile([P, RT], F32)
            biasp = small.tile([P, RT], F32)
            ed4 = small.tile([P, RT], F32)

            # critical-first load order; big tensors split per chunk so the
            # first matmuls start after ~256KB instead of ~1MB
            nc.sync.dma_start(out=eah_sb[:], in_=eah[:])
            nc.sync.dma_start(out=eal_sb[:], in_=eal[:])
            nc.sync.dma_start(
                out=meta_sb[:], in_=meta.rearrange("(r p) k -> p r k", p=P)
            )
            for c in range(NCH):
                cs = slice(c * CHW, (c + 1) * CHW)
                nc.sync.dma_start(out=eth_sb[:, cs], in_=eth[:, cs])
                nc.sync.dma_start(out=etl_sb[:, cs], in_=etl[:, cs])
            nc.sync.dma_start(out=ebh_sb[:], in_=ebh[:])
            nc.sync.dma_start(out=ebl_sb[:], in_=ebl[:])
            nc.vector.memset(ones, 1.0)
            nc.gpsimd.iota(
                iota_sb[:], [[1, CHW]], base=0, channel_multiplier=0,
                allow_small_or_imprecise_dtypes=True,
            )

            import contextlib
            loop_cm = (
                tc.For_i(0, bench_iters, 1) if bench_iters > 1
                else contextlib.nullcontext()
            )
            with loop_cm:
              for rt in range(RT):
                mrt = meta_sb[:, rt]
                lh = eah_sb[:, rt * P:(rt + 1) * P]
                ll = eal_sb[:, rt * P:(rt + 1) * P]
                # --- positives first: penalty tile on (otherwise idle) GPSIMD
                pen1 = bandp.tile([P, BW], F32, tag="pen1")
                pen2 = bandp.tile([P, BW], F32, tag="pen2")
                pen = bandp.tile([P, BW], F32, tag="pen")
                nc.gpsimd.tensor_scalar(
                    out=pen1[:], in0=iota_sb[:, :BW],
                    scalar1=mrt[:, 16:17], scalar2=PEN,
                    op0=ALU.is_lt, op1=ALU.mult,
                )
                nc.gpsimd.tensor_scalar(
                    out=pen2[:], in0=iota_sb[:, :BW],
                    scalar1=mrt[:, 17:18], scalar2=PEN,
                    op0=ALU.is_ge, op1=ALU.mult,
                )
                nc.gpsimd.tensor_add(pen[:], pen1[:], pen2[:])
                pbt = psum_s.tile([P, CHW], F32, tag="ps")
                pb = pbt[:, :BW]
                nc.tensor.matmul(
                    pb, lh, ebh_sb[:, rt * BW:(rt + 1) * BW],
                    start=True, stop=False,
                )
                nc.tensor.matmul(
                    pb, lh, ebl_sb[:, rt * BW:(rt + 1) * BW],
                    start=False, stop=False,
                )
                nc.tensor.matmul(
                    pb, ll, ebh_sb[:, rt * BW:(rt + 1) * BW],
                    start=False, stop=True,
                )
                bp = bandp.tile([P, BW], F32, tag="bp")
                nc.vector._custom_dve(
                    CIRCLE_POS,
                    out=bp[:], in0=pb, in1=pen[:],
                    s1=SQRT80, imm2=CLAMP_P,
                    accum_out=mxp1[:, rt:rt + 1],
                )
                # --- negatives: full row in 4-bank superchunks, term-major
                # matmul order so LDWEIGHTS switches only twice per chunk
                nhalves = CHW // CH
                for c in range(NCH):
                    ps = psum_s.tile([P, CHW], F32, tag="ps")
                    for ti, (w, rhs_sb) in enumerate(
                        [(lh, eth_sb), (lh, etl_sb), (ll, eth_sb)]
                    ):
                        for half in range(nhalves):
                            cs = c * CHW + half * CH
                            dst = ps[:, half * CH:(half + 1) * CH]
                            nc.tensor.matmul(
                                dst, w, rhs_sb[:, cs:cs + CH],
                                start=(ti == 0), stop=(ti == 2),
                            )
                    nc.vector._custom_dve(
                        CIRCLE_NEG,
                        out=sqm_all[:, rt, c * CHW:(c + 1) * CHW],
                        in0=ps[:],
                        in1=iota_sb[:],
                        s0=mrt[:, c:c + 1],
                        s1=mrt[:, 8:9],
                        imm2=CLAMP_N,
                        accum_out=mxn4[:, rt, c:c + 1],
                    )

                # --- per-row-tile shifts (tiny) so ACT can start
                nc.vector.tensor_reduce(
                    biasn[:, rt:rt + 1], mxn4[:, rt], axis=mybir.AxisListType.X,
                    op=ALU.max, negate=True,
                )
                nc.vector.tensor_scalar_mul(
                    biasp[:, rt:rt + 1], mxp1[:, rt:rt + 1], -1.0
                )
                # --- exp + row sums (ACT, two half-row passes); masked -> 0
                hw = B // 2
                for hf in range(2):
                    esc = escp.tile([P, hw], F32, tag="esc")
                    nc.scalar.activation(
                        out=esc[:],
                        in_=sqm_all[:, rt, hf * hw:(hf + 1) * hw],
                        func=AF.Exp, bias=biasn[:, rt:rt + 1], scale=1.0,
                        accum_out=sn2[:, rt, hf:hf + 1],
                    )
                escb = escp.tile([P, BW], F32, tag="escb")
                nc.scalar.activation(
                    out=escb[:], in_=bp[:],
                    func=AF.Exp, bias=biasp[:, rt:rt + 1], scale=1.0,
                    accum_out=sp4[:, rt:rt + 1],
                )
                # diagonal's exp contribution (to subtract later)
                nc.scalar.activation(
                    out=ed4[:, rt:rt + 1], in_=mrt[:, 18:19],
                    func=AF.Exp, bias=biasp[:, rt:rt + 1], scale=1.0,
                )

            # ---- batched per-anchor tail on [P, RT] tiles (all tiny)
            nc.vector.tensor_reduce(
                sn4[:], sn2[:], axis=mybir.AxisListType.X, op=ALU.add
            )
            lnn = small.tile([P, RT], F32)
            nc.vector.tensor_scalar_add(sn4[:], sn4[:], 1e-30)
            nc.scalar.activation(out=lnn[:], in_=sn4[:], func=AF.Ln)

            # positives: subtract diagonal, guard at 0, log
            nc.vector.tensor_sub(sp4[:], sp4[:], ed4[:])
            nc.vector.tensor_scalar(
                out=sp4[:], in0=sp4[:], scalar1=0.0, scalar2=1e-30,
                op0=ALU.max, op1=ALU.add,
            )
            lnp = small.tile([P, RT], F32)
            nc.scalar.activation(out=lnp[:], in_=sp4[:], func=AF.Ln)

            z = small.tile([P, RT], F32)
            # z = (mxn + mxp) + (lnn + lnp) + cnt ; mx* = -bias*
            nc.vector.tensor_add(z[:], biasn[:], biasp[:])
            nc.vector.tensor_scalar_mul(z[:], z[:], -1.0)
            nc.vector.tensor_add(z[:], z[:], lnn[:])
            nc.vector.tensor_add(z[:], z[:], lnp[:])
            nc.vector.tensor_add(z[:], z[:], meta_sb[:, :, 20])

            # stable softplus: relu(z) + softplus(-|z|) (one ACT table func)
            rl = small.tile([P, RT], F32)
            nc.gpsimd.tensor_scalar_max(rl[:], z[:], 0.0)
            negz = small.tile([P, RT], F32)
            nc.gpsimd.tensor_scalar_mul(negz[:], z[:], -1.0)
            ab = small.tile([P, RT], F32)
            nc.vector.tensor_max(ab[:], z[:], negz[:])
            # exp(-|z|) via Schraudolph bit trick on DVE (avoids an ACT
            # Exp<->Ln table reload; |error| ~3% of a term bounded by ln 2,
            # and +inf/huge |z| map to exactly 0)
            ey = small.tile([P, RT], F32)
            nc.vector.tensor_scalar(
                out=ey[:], in0=ab[:], scalar1=-12102203.0, scalar2=1064866805.0,
                op0=ALU.mult, op1=ALU.add,
            )
            nc.vector.tensor_scalar_max(ey[:], ey[:], 0.0)
            eyi = small.tile([P, RT], mybir.dt.int32)
            nc.vector.tensor_copy(eyi[:], ey[:])
            enx = small.tile([P, RT], F32)
            nc.vector.tensor_scalar_add(enx[:], eyi[:].bitcast(F32), 1.0)
            l1p = small.tile([P, RT], F32)
            nc.scalar.activation(out=l1p[:], in_=enx[:], func=AF.Ln)
            sp = small.tile([P, RT], F32)
            nc.vector.tensor_add(sp[:], rl[:], l1p[:])
            nc.vector.tensor_mul(sp[:], sp[:], meta_sb[:, :, 21])

            tot = small.tile([P, 1], F32)
            nc.vector.tensor_reduce(
                tot[:], sp[:], axis=mybir.AxisListType.X, op=ALU.add
            )
            pf = psum_f.tile([1, 1], F32, tag="pf")
            nc.tensor.matmul(pf[:], tot[:], ones[:], start=True, stop=True)
            osb = small.tile([1, 1], F32)
            nc.vector.tensor_copy(osb[:], pf[:])
            nc.sync.dma_start(out=out[:], in_=osb[:])

    nc.compile()
    return nc


# ---------------------------------------------------------------------------
# Host side
# ---------------------------------------------------------------------------


def host_prep(E, labels, batch_size):
    order = np.argsort(labels, kind="stable")
    labels_s = labels[order]
    idx = np.arange(B)
    keep = ((idx % 4 == 0) & (idx < batch_size)) | (idx > batch_size)
    keep_s = keep[order]

    change = np.empty(B, bool)
    change[0] = True
    change[1:] = labels_s[1:] != labels_s[:-1]
    firsts = np.flatnonzero(change)
    bounds = np.concatenate([firsts, [B]])
    start = np.repeat(bounds[:-1], np.diff(bounds))
    end = np.repeat(bounds[1:], np.diff(bounds))

    gsize = end - start
    p_cnt = gsize - 1
    n_cnt = B - gsize
    valid = keep_s & (p_cnt > 0) & (n_cnt > 0)
    cnt = (
        np.log(np.maximum(p_cnt, 1)) + np.log(np.maximum(n_cnt, 1)) - 25.6
    ).astype(np.float32)
    n_valid = int(valid.sum())

    # pre-scaled, transposed embeddings [D, B]
    E_T = np.ascontiguousarray(
        E[order].T * np.float32(SCALE_E), dtype=np.float32
    )
    return E_T, start, end, valid, cnt, n_valid


def band_width_needed(start, end):
    w = 0
    for core in range(NCORES):
        a0 = core * APC
        for rt in range(RT):
            lo = int(start[a0 + rt * P])
            hi = int(end[a0 + rt * P + P - 1])
            w = max(w, hi - lo)
    for bw in BW_CHOICES:
        if w <= bw:
            return bw
    raise ValueError(f"band width {w} exceeds {BW_CHOICES[-1]}")


def make_core_inputs(E_T, start, end, valid, cnt, core, BW):
    a0 = core * APC
    st = start[a0:a0 + APC]
    en = end[a0:a0 + APC]

    center = (st + en - 1).astype(np.float64) / 2.0
    half = (en - st - 1).astype(np.float64) / 2.0

    band_start = np.zeros(APC, np.int64)
    for rt in range(RT):
        lo = int(st[rt * P])
        hi = int(en[rt * P + P - 1])
        if hi - lo > BW:
            raise ValueError(f"band width {hi - lo} exceeds {BW}")
        bs = min(max(lo, 0), B - BW)
        band_start[rt * P:(rt + 1) * P] = bs

    # diagonal value: S'_aa = sum of scaled e^2 (f32), clamped transform
    ecols = E_T[:, a0:a0 + APC].astype(np.float32)
    s_aa = np.sum(ecols * ecols, axis=0, dtype=np.float32)
    dsq = np.minimum(
        s_aa - np.float32(SQRT80), np.float32(CLAMP_P)
    ).astype(np.float32) ** 2

    meta = np.zeros((APC, MCOLS), np.float32)
    for c in range(NCH):
        meta[:, c] = (center - CHW * c).astype(np.float32)
    meta[:, 8] = half.astype(np.float32)
    meta[:, 16] = (st - band_start).astype(np.float32)
    meta[:, 17] = (en - band_start).astype(np.float32)
    meta[:, 18] = dsq
    meta[:, 20] = cnt[a0:a0 + APC]
    meta[:, 21] = valid[a0:a0 + APC].astype(np.float32)

    eb = np.empty((P, RT * BW), np.float32)
    for rt in range(RT):
        bs = int(band_start[rt * P])
        eb[:, rt * BW:(rt + 1) * BW] = E_T[:, bs:bs + BW]

    def split16(a):
        h = a.astype(np.float16)
        l = (a - h.astype(np.float32)).astype(np.float16)
        return h, l

    eth, etl = split16(E_T)
    ea = np.ascontiguousarray(E_T[:, a0:a0 + APC])
    eah, eal = split16(ea)
    ebh, ebl = split16(eb)

    return {
        "eth": eth,
        "etl": etl,
        "eah": eah,
        "eal": eal,
        "ebh": ebh,
        "ebl": ebl,
        "meta": meta,
    }


_PROGRAM_CACHE = {}


def _get_program(BW=384):
    key = ("nc", BW)
    if key not in _PROGRAM_CACHE:
        _PROGRAM_CACHE[key] = build_program(BW)
    return _PROGRAM_CACHE[key]


def _build_executor(nc, n_cores=NCORES):
    """Persistent jitted runner (mirrors bass2jax.run_bass_via_pjrt's
    multi-core branch) so repeated kernel() calls skip jax re-tracing."""
    import jax
    from jax.experimental.shard_map import shard_map
    from jax.sharding import Mesh, PartitionSpec
    from concourse import bass2jax
    from concourse import mybir as _mb

    bass2jax.install_neuronx_cc_hook()
    partition_name = (
        nc.partition_id_tensor.name if nc.partition_id_tensor else None
    )
    in_names, out_names, out_avals, zero_templates = [], [], [], []
    for alloc in nc.m.functions[0].allocations:
        if not isinstance(alloc, _mb.MemoryLocationSet):
            continue
        name = alloc.memorylocations[0].name
        if alloc.kind == "ExternalInput":
            if name != partition_name:
                in_names.append(name)
        elif alloc.kind == "ExternalOutput":
            shape = tuple(alloc.tensor_shape)
            dtype = _mb.dt.np(alloc.dtype)
            out_names.append(name)
            out_avals.append(jax.core.ShapedArray(shape, dtype))
            zero_templates.append((shape, dtype))
    n_params = len(in_names)
    n_outs = len(out_avals)
    all_names = list(in_names) + list(out_names)
    if partition_name is not None:
        all_names.append(partition_name)
    donate = tuple(range(n_params, n_params + n_outs))

    def _body(*args):
        operands = list(args)
        if partition_name is not None:
            operands.append(bass2jax.partition_id_tensor())
        outs = bass2jax._bass_exec_p.bind(
            *operands,
            out_avals=tuple(out_avals),
            in_names=tuple(all_names),
            out_names=tuple(out_names),
            lowering_input_output_aliases=(),
            sim_require_finite=True,
            sim_require_nnan=True,
            nc=nc,
        )
        return tuple(outs)

    devices = jax.devices()[:n_cores]
    mesh = Mesh(np.asarray(devices), ("core",))
    # "et" is identical on every core -> replicate instead of concatenating
    # 8 copies (saves ~14MB of host->device traffic per call).
    replicated = {"eth", "etl"}
    in_specs = tuple(
        PartitionSpec() if name in replicated else PartitionSpec("core")
        for name in in_names
    ) + (PartitionSpec("core"),) * n_outs
    out_specs = (PartitionSpec("core"),) * n_outs
    sharded = jax.jit(
        shard_map(_body, mesh=mesh, in_specs=in_specs, out_specs=out_specs,
                  check_rep=False),
        donate_argnums=donate, keep_unused=True,
    )

    from jax.sharding import NamedSharding

    def place(in_maps):
        """Device-put the call inputs once; returns the positional arg list
        (without the donated zero buffers)."""
        arrs = []
        for name in in_names:
            if name in replicated:
                a = np.asarray(in_maps[0][name])
                sh = NamedSharding(mesh, PartitionSpec())
            else:
                a = np.concatenate(
                    [np.asarray(m[name]) for m in in_maps], axis=0
                )
                sh = NamedSharding(mesh, PartitionSpec("core"))
            arrs.append(jax.device_put(a, sh))
        return arrs

    zero_sharding = NamedSharding(mesh, PartitionSpec("core"))

    def exec_async(dev_in):
        concat_zeros = [
            jax.device_put(np.zeros((n_cores * s[0], *s[1:]), dt), zero_sharding)
            for s, dt in zero_templates
        ]
        return sharded(*dev_in, *concat_zeros)

    def run(in_maps):
        out_arrs = exec_async(place(in_maps))
        return [
            {
                name: np.asarray(out_arrs[i]).reshape(n_cores, *out_avals[i].shape)[c]
                for i, name in enumerate(out_names)
            }
            for c in range(n_cores)
        ]

    run.place = place
    run.exec_async = exec_async
    return run


def _get_executor(BW=384):
    key = ("exec", BW)
    if key not in _PROGRAM_CACHE:
        nc = _get_program(BW)
        try:
            _PROGRAM_CACHE[key] = _build_executor(nc)
        except Exception:
            _PROGRAM_CACHE[key] = None
    return _PROGRAM_CACHE[key]


def _run_device(in_maps, BW=384):
    ex = _get_executor(BW)
    if ex is not None:
        try:
            return ex(in_maps)
        except Exception:
            _PROGRAM_CACHE[("exec", BW)] = None
    res = run_bass_kernel_spmd(
        _get_program(BW), in_maps, core_ids=list(range(NCORES))
    )
    return res.results


def make_all_inputs(embeddings, labels, batch_size):
    E = np.asarray(embeddings, np.float32)
    labels_np = np.asarray(labels).astype(np.int64).reshape(-1)
    bs = int(np.asarray(batch_size).reshape(()))
    assert E.shape == (B, D)
    E_T, start, end, valid, cnt, n_valid = host_prep(E, labels_np, bs)
    BW = band_width_needed(start, end)
    in_maps = [
        make_core_inputs(E_T, start, end, valid, cnt, c, BW)
        for c in range(NCORES)
    ]
    return in_maps, n_valid, BW


def kernel(embeddings, labels, batch_size):
    in_maps, n_valid, BW = make_all_inputs(embeddings, labels, batch_size)
    results = _run_device(in_maps, BW)
    partials = [float(r["out"][0, 0]) for r in results]
    loss = np.float32(math.fsum(partials) / max(n_valid, 1))
    return np.asarray(loss, dtype=np.float32)
